# revision 23
# baseline (speedup 1.0000x reference)
"""ASSR reconstruction loss on 8 Trainium2 NeuronCores.

total = mean|pred-target| + 0.1 * (mean|bicubic_aa_resize(pred, 128,128) - lr_ref|)

Data-parallel over the batch axis: core i handles batches [4i, 4i+4).
Each core returns raw per-partition accumulator columns (one per L1
piece); the host combines them into the 5 reference outputs.

Schedule: the DMA stream is the bottleneck (24.75 MiB/rep per core at
~360 B/ns), so the kernel software-pipelines compute against it:

- pred loads lead targ loads by `pred_lead` images; the resize matmul
  chain for image i is issued at i's pred step (pred-gated, runs early);
  all L1 work is issued at i's targ step (targ-gated, runs late). Every
  engine queue therefore sees targ-gated instructions only behind
  pred-gated ones, so nothing late blocks anything early.
- the lr L1 subtract is folded into the stage-2 matmul by seeding its
  PSUM bank with -lr (DVE) and accumulating the resize on top
  (start=False), so the lr term costs no tail tensor_tensor.
- the last two images' targ loads are chunked and each piece's
  |pred-targ| is spread across DVE/Pool/ACT per a tail plan, so the
  ~10us of L1 engine work that lands in the final ~6us of stream is
  absorbed by three lanes instead of backing up one.
"""

import numpy as np
from contextlib import ExitStack

import concourse.bass as bass
import concourse.bacc as bacc
import concourse.tile as tile
import concourse.mybir as mybir
from concourse.bass_utils import run_bass_kernel_spmd

F32 = mybir.dt.float32
AF = mybir.ActivationFunctionType
ALU = mybir.AluOpType
AX = mybir.AxisListType

N_CORES = 8
B, C, H, W = 32, 3, 512, 512
TH, TW = 128, 128
BS = B // N_CORES         # batches per core
IMGS = BS * C             # images per core
HC = H // 128             # h chunks of 128
WC = W // 128             # w chunks of 128

# tail piece plans: the last TAIL_N images' targ loads are chunked and each
# piece's |pred-targ| is assigned an engine strategy:
#   a = DVE sub + ACT abs-accum       (1 plus col)
#   b = Pool sub + ACT abs-accum      (1 plus col)
#   c = DVE ttr max-sum + min-sum     (1 plus + 1 minus col; host subtracts)
#   e = DVE sub + DVE abs-reduce      (1 plus col; shortest latency)
#   f = Pool sub + DVE abs-reduce     (1 plus col)
# a tail image with a 4-letter plan is processed as 4 h-chunk pieces; an
# 8-letter plan splits every h-chunk into two W-halves (finer landing
# granularity). The number of tail images is the length of the plan tuple.
# g1/g2 come from a greedy list-scheduler over the measured per-op costs.
TAIL_PLANS = {
    "q1": (list("cbcb"), list("bcbc"), list("cbaae")),
    "t3": (list("bbfb"), list("acab"), list("cbab"), list("acfae")),
    "n3": (list("bbfb"), list("acab"), list("cbab"), list("bcfbe")),
    "noct": (list("aaaa"), list("aaaa"), list("aaaa"), list("aaaae")),
    "x12b": (list("fbeb"), list("ebbb"), list("ebeb"), list("ebebe")),
    "noc": (list("bbab"), list("abab"), list("bbab"), list("abfbe")),
    "nob": (list("caca"), list("acac"), list("caca"), list("accae")),
    "g2": (list("fbcb"), list("cbac"), list("aacbf")),
    "g1": (list("ffffccbb"), list("cbbcbbcb"), list("acbcbcbc")),
    "g4": (list("ffffccbb"), list("cbabcbcb"), list("cbaacbaf")),
    "g3": (list("ffccbbcb"), list("bcbbcbac"), list("bcbcbcbc")),
}


def _plan_cols(plan):
    nplus = IMGS - len(plan) + sum(len(p) for p in plan)
    nminus = sum(s == "c" for p in plan for s in p)
    return nplus, nminus


LAM_CONSIST = 0.1
LAM_LR = 1.0

DEFAULT_OPTS = dict(
    pred_lead=3,         # images the pred stream leads the targ stream by
    bufs=6,              # pred pool depth (>= pred_lead + 2)
    tbufs=4,             # targ pool depth
    d_bufs=5,            # diff-tile pool depth
    lr_bufs=4,           # lr pool depth (all batches resident)
    t1t_bufs=3,          # stage-1 PSUM depth (decouples PE from ACT copy)
    copy_eng="dve",      # engine for t1t/out2 PSUM->SBUF copies (dve/act;
                         # pool cannot touch PSUM on this compiler)
    defer_s2=False,      # issue image j's copy/seed/stage-2 at pred step j+1
    out2_bufs=4,         # stage-2 PSUM depth (freed early by the DVE copy)
    tail_plan="x12b",    # TAIL_PLANS key
    compute=True,        # False = DMA-only skeleton (for floor measurement)
    reps=1,              # replicate the main loop (for HW timing deltas)
    timing_dram=False,   # image tensors are Internal DRAM scratch (no host xfer)
    split_rings=False,   # issue targ loads from the ACT HWDGE ring
    lr_swdge=False,      # issue lr_ref loads from the gpsimd SWDGE queue
)


def _cubic(x, a=-0.75):
    # float32, mirrors the reference's PyTorch bicubic kernel
    ax = np.abs(x)
    ax2 = ax * ax
    ax3 = ax2 * ax
    f1 = (a + 2.0) * ax3 - (a + 3.0) * ax2 + 1.0
    f2 = a * ax3 - 5.0 * a * ax2 + 8.0 * a * ax - 4.0 * a
    return np.where(ax <= 1.0, f1, np.where(ax < 2.0, f2, np.float32(0.0)))


def _resize_matrix(in_size: int, out_size: int) -> np.ndarray:
    scale = in_size / out_size
    s_aa = max(scale, 1.0)
    support = 2.0 * s_aa
    ext = int(np.ceil(support)) + 1
    centers = (np.arange(out_size, dtype=np.float64) + 0.5) * scale - 0.5
    idx = np.arange(-ext, in_size + ext)
    dist = (idx[None, :] - centers[:, None]) / s_aa
    w = _cubic(dist.astype(np.float32)).astype(np.float32)
    w = w / np.sum(w, axis=1, keepdims=True)
    idx_c = np.clip(idx, 0, in_size - 1)
    M = np.zeros((out_size, in_size), dtype=np.float32)
    np.add.at(M, (np.arange(out_size)[:, None], idx_c[None, :].repeat(out_size, 0)), w)
    return M


_CACHE = {}


def _build(**opts):
    o = {**DEFAULT_OPTS, **opts}
    key = tuple(sorted(o.items()))
    if key in _CACHE:
        return _CACHE[key]

    plan = TAIL_PLANS[o["tail_plan"]]
    tail_n = len(plan)
    nplus, nminus = _plan_cols(plan)
    ncols = nplus + nminus + IMGS   # [plus | minus | lr]

    nc = bacc.Bacc("TRN2", target_bir_lowering=False, debug=False,
                   num_devices=N_CORES)
    img_kind = "Internal" if o["timing_dram"] else "ExternalInput"
    pred_d = nc.dram_tensor("pred", [BS, C, H, W], F32, kind=img_kind)
    targ_d = nc.dram_tensor("targ", [BS, C, H, W], F32, kind=img_kind)
    lr_d = nc.dram_tensor("lr", [BS, C, TH, TW], F32, kind=img_kind)
    mT_d = nc.dram_tensor("mT", [H, TH], F32, kind="ExternalInput")  # M^T
    negI_d = nc.dram_tensor("negI", [TH, TH], F32, kind="ExternalInput")
    # raw per-partition accumulator columns; host does the final sums.
    out_d = nc.dram_tensor("acc_out", [128, ncols], F32, kind="ExternalOutput")

    # partition p holds rows 4p..4p+3 of the image: fully contiguous 8KB
    pred_v = pred_d.ap().rearrange("b c (p r) w -> b c p r w", p=128)
    targ_v = targ_d.ap().rearrange("b c (p r) w -> b c p r w", p=128)
    # stage-1 matmul contracts h = 4p + r
    mT1_v = mT_d.ap().rearrange("(p r) o -> p r o", p=128)
    lr_v = lr_d.ap().rearrange("b c p w -> b p c w")          # [b, 128, C, 128]
    mT2_v = mT_d.ap().rearrange("(wc p) o -> p wc o", p=128)  # [128, WC, 128]

    targ_dma = nc.scalar if o["split_rings"] else nc.sync
    lr_dma = nc.gpsimd if o["lr_swdge"] else nc.sync

    with tile.TileContext(nc) as tc, ExitStack() as ctx:
        consts = ctx.enter_context(tc.tile_pool(name="consts", bufs=1))
        accs = ctx.enter_context(tc.tile_pool(name="accs", bufs=1))
        pred_p = ctx.enter_context(tc.tile_pool(name="pred_p", bufs=o["bufs"]))
        targ_p = ctx.enter_context(tc.tile_pool(name="targ_p", bufs=o["tbufs"]))
        d_p = ctx.enter_context(tc.tile_pool(name="d_p", bufs=o["d_bufs"]))
        sinks = ctx.enter_context(tc.tile_pool(name="sinks", bufs=1))
        t1t_p = ctx.enter_context(tc.tile_pool(name="t1t_p", bufs=3))
        out2sb_p = ctx.enter_context(tc.tile_pool(name="out2sb_p", bufs=5))
        lr_p = ctx.enter_context(tc.tile_pool(name="lr_p", bufs=o["lr_bufs"]))
        ps_t1t = ctx.enter_context(
            tc.tile_pool(name="ps_t1t", bufs=o["t1t_bufs"], space="PSUM"))
        ps_out2 = ctx.enter_context(
            tc.tile_pool(name="ps_out2", bufs=o["out2_bufs"], space="PSUM"))

        mT1_t = consts.tile([128, HC, TH], F32)
        nc.sync.dma_start(mT1_t[:], mT1_v)
        mT2_t = consts.tile([128, WC, TH], F32)
        nc.sync.dma_start(mT2_t[:], mT2_v)
        negI_t = consts.tile([128, TH], F32)
        nc.sync.dma_start(negI_t[:], negI_d.ap())

        acc = accs.tile([128, ncols], F32)
        # write-only outputs of abs/ttr ops: only the accum column matters,
        # and each sink is written by exactly one (in-order) engine, so WAW
        # on a shared sink imposes no extra ordering.
        act_sink = sinks.tile([128, HC, W], F32)
        dve_sink = sinks.tile([128, HC, W], F32)
        plus_i = iter(range(nplus))
        minus_i = iter(range(nplus, nplus + nminus))

        def resize_s1(pred_im):
            """stage-1 matmuls only (PE, pred-gated)."""
            t1t_ps = ps_t1t.tile([128, WC, TH], F32, tag="t1t_ps")
            for wc in range(WC):
                for k in range(HC):
                    nc.tensor.matmul(
                        t1t_ps[:, wc, :],
                        lhsT=pred_im[:, k, bass.ts(wc, 128)],
                        rhs=mT1_t[:, k, :],
                        start=(k == 0), stop=(k == HC - 1),
                    )
            return t1t_ps

        def copy_op(dst, src):
            # PSUM->SBUF copies: DVE by default; Pool keeps both DVE and ACT
            # free (gpsimd reads PSUM fine); ACT couples PE to the abs chain
            if o["copy_eng"] == "act":
                nc.scalar.copy(dst, src)
            elif o["copy_eng"] == "pool":
                nc.gpsimd.tensor_scalar_add(dst, src, 0.0)
            else:
                nc.vector.tensor_scalar_add(dst, src, 0.0)

        def resize_finish(t1t_ps, lr_col):
            """PSUM copy + (-lr matmul seed) + stage-2 + PSUM drain.
            Returns the SBUF tile holding resize(pred) - lr."""
            t1t_t = t1t_p.tile([128, WC, TH], F32, tag="t1t")
            copy_op(t1t_t[:], t1t_ps[:])
            out2_ps = ps_out2.tile([128, TW], F32, tag="out2")
            # seed the accumulation group with -lr via a (-I) matmul: PSUM
            # groups must be started by the PE, and this costs no vector-
            # engine time (PE has slack)
            nc.tensor.matmul(out2_ps[:], lhsT=negI_t[:], rhs=lr_col,
                             start=True, stop=False)
            for wc in range(WC):
                nc.tensor.matmul(
                    out2_ps[:],
                    lhsT=t1t_t[:, wc, :],
                    rhs=mT2_t[:, wc, :],
                    start=False, stop=(wc == WC - 1),
                )
            # drain PSUM to SBUF immediately (pred-gated) so the PSUM bank
            # recycles without waiting on ACT's targ-gated abs backlog
            out2_sb = out2sb_p.tile([128, TW], F32, tag="out2sb")
            copy_op(out2_sb[:], out2_ps[:])
            return out2_sb

        def lr_l1(i, out2_sb):
            col = nplus + nminus + i
            nc.scalar.activation(act_sink[:, 0, 0:TW], out2_sb[:], AF.Abs,
                                 accum_out=acc[:, col:col + 1])

        def pix_mid(pred_im, targ_im):
            """whole-image |pred - targ|: DVE sub, ACT abs-accum."""
            c = next(plus_i)
            d_t = d_p.tile([128, HC, W], F32, tag="d")
            nc.vector.tensor_sub(d_t[:], pred_im[:], targ_im[:])
            nc.scalar.activation(act_sink[:], d_t[:], AF.Abs,
                                 accum_out=acc[:, c:c + 1])

        def pix_piece(d_t, sl, pred_im, targ_im, strat):
            if strat == "c":
                cp = next(plus_i)
                cm = next(minus_i)
                nc.vector.tensor_tensor_reduce(
                    dve_sink[sl], pred_im[sl], targ_im[sl], 1.0, 0.0,
                    ALU.max, ALU.add, acc[:, cp:cp + 1])
                nc.vector.tensor_tensor_reduce(
                    dve_sink[sl], pred_im[sl], targ_im[sl], 1.0, 0.0,
                    ALU.min, ALU.add, acc[:, cm:cm + 1])
                return
            cp = next(plus_i)
            col = acc[:, cp:cp + 1]
            eng = nc.gpsimd if strat in "bf" else nc.vector
            eng.tensor_sub(d_t[sl], pred_im[sl], targ_im[sl])
            if strat in "ef":
                nc.vector.tensor_reduce(col, d_t[sl], axis=AX.XY, op=ALU.add,
                                        apply_absolute_value=True)
            else:
                nc.scalar.activation(act_sink[sl], d_t[sl], AF.Abs,
                                     accum_out=col)

        def tail_pieces(i):
            """(slice, strat) list for tail image i. A 4-letter plan means
            h-chunk pieces (the last image's final chunk W-split in 2, using
            the 5th letter); an 8-letter plan W-splits every h-chunk."""
            strats = plan[i - (IMGS - tail_n)]
            half = W // 2
            sls = []
            if len(strats) == 8:
                for j in range(HC):
                    for h in range(2):
                        sls.append((slice(None), slice(j, j + 1),
                                    slice(h * half, (h + 1) * half)))
            else:
                nfull = HC - 1 if len(strats) == 5 else HC
                for j in range(nfull):
                    sls.append((slice(None), slice(j, j + 1), slice(None)))
                if len(strats) == 5:
                    for h in range(2):
                        sls.append((slice(None), slice(HC - 1, HC),
                                    slice(h * half, (h + 1) * half)))
            assert len(sls) == len(strats)
            return list(zip(sls, strats))

        L = min(o["pred_lead"], IMGS)
        for _rep in range(o["reps"]):
            plus_i = iter(range(nplus))
            minus_i = iter(range(nplus, nplus + nminus))
            pred_tiles = {}
            out2_tiles = {}
            lr_tiles = {}

            t1t_pending = {}

            def finish(j):
                t1t_ps, lr_col = t1t_pending.pop(j)
                out2_tiles[j] = resize_finish(t1t_ps, lr_col)

            def pred_step(j):
                b, c = divmod(j, C)
                if c == 0:
                    lr_t = lr_p.tile([128, C, TW], F32, tag="lr")
                    lr_dma.dma_start(lr_t[:], lr_v[b])
                    lr_tiles[b] = lr_t
                pt = pred_p.tile([128, HC, W], F32, tag="pred")
                nc.sync.dma_start(pt[:], pred_v[b, c])
                pred_tiles[j] = pt
                if o["compute"]:
                    t1t_pending[j] = (resize_s1(pt[:]), lr_tiles[b][:, c, :])
                    if o["defer_s2"]:
                        # issue image j-1's copy/stage-2 now: its t1t copy is
                        # ready (s1_{j-1} done while s1_j streams), so PE
                        # never bubbles waiting on a cross-engine copy
                        if j - 1 in t1t_pending:
                            finish(j - 1)
                        if j == IMGS - 1:
                            finish(j)
                    else:
                        finish(j)

            for j in range(L):
                pred_step(j)
            for i in range(IMGS):
                if i + L < IMGS:
                    pred_step(i + L)
                b, c = divmod(i, C)
                tail = (i >= IMGS - tail_n)
                targ_t = targ_p.tile([128, HC, W], F32, tag="targ")
                if tail:
                    pieces = tail_pieces(i)
                    # DMA stays at h-chunk granularity (HWDGE's fixed
                    # per-DMA cost caps ~4 DMAs per image-slot); an 8-piece
                    # plan just computes two W-halves per landed chunk
                    for j in range(HC):
                        sl = (slice(None), slice(j, j + 1), slice(None))
                        targ_dma.dma_start(targ_t[sl], targ_v[b, c][sl])
                else:
                    targ_dma.dma_start(targ_t[:], targ_v[b, c])
                if not o["compute"]:
                    pred_tiles.pop(i)
                    continue
                pred_im = pred_tiles.pop(i)
                lr_l1(i, out2_tiles.pop(i))
                if tail:
                    d_t = d_p.tile([128, HC, W], F32, tag="d")
                    for sl, strat in pieces:
                        pix_piece(d_t, sl, pred_im[:], targ_t[:], strat)
                else:
                    pix_mid(pred_im[:], targ_t[:])

        if not o["compute"]:
            nc.vector.memset(acc[:], 0.0)
        # single store: post-wait DMA latency is fixed, so splitting buys
        # nothing; the SEQ sits waiting on the last piece's reduce only.
        nc.sync.dma_start(out_d.ap()[:, :], acc[:])

    nc.compile()
    _CACHE[key] = nc
    return nc


def _make_runner(nc):
    """Build the sharded PJRT callable once (mirrors bass2jax.run_bass_via_pjrt,
    but reusable across calls so repeat invocations skip retrace/NEFF reload)."""
    import jax
    from jax.sharding import Mesh, PartitionSpec
    from jax.experimental.shard_map import shard_map
    from concourse.bass2jax import (_bass_exec_p, install_neuronx_cc_hook,
                                    partition_id_tensor)

    install_neuronx_cc_hook()
    in_names, out_names, out_avals = [], [], []
    for alloc in nc.m.functions[0].allocations:
        if not isinstance(alloc, mybir.MemoryLocationSet):
            continue
        name = alloc.memorylocations[0].name
        if alloc.kind == "ExternalInput":
            if nc.partition_id_tensor is None or name != nc.partition_id_tensor.name:
                in_names.append(name)
        elif alloc.kind == "ExternalOutput":
            out_names.append(name)
            out_avals.append(jax.core.ShapedArray(
                tuple(alloc.tensor_shape), mybir.dt.np(alloc.dtype)))
    n_params = len(in_names)
    n_outs = len(out_avals)
    all_names = list(in_names) + out_names
    if nc.partition_id_tensor is not None:
        all_names.append(nc.partition_id_tensor.name)

    def _body(*args):
        operands = list(args)
        if nc.partition_id_tensor is not None:
            operands.append(partition_id_tensor())
        return tuple(_bass_exec_p.bind(
            *operands,
            out_avals=tuple(out_avals),
            in_names=tuple(all_names),
            out_names=tuple(out_names),
            lowering_input_output_aliases=(),
            sim_require_finite=True,
            sim_require_nnan=True,
            nc=nc,
        ))

    devices = jax.devices()[:N_CORES]
    mesh = Mesh(np.asarray(devices), ("core",))
    in_specs = (PartitionSpec("core"),) * (n_params + n_outs)
    out_specs = (PartitionSpec("core"),) * n_outs
    sharded = jax.jit(
        shard_map(_body, mesh=mesh, in_specs=in_specs, out_specs=out_specs,
                  check_rep=False),
        keep_unused=True,
    )

    def run_concat(concat_by_name):
        """concat_by_name: input name -> global array (cores stacked on axis 0)."""
        concat_in = [np.ascontiguousarray(concat_by_name[name]) for name in in_names]
        zeros = [np.zeros((N_CORES * a.shape[0], *a.shape[1:]), a.dtype)
                 for a in out_avals]
        out_arrs = sharded(*concat_in, *zeros)
        return [
            {name: np.asarray(out_arrs[i]).reshape(N_CORES, *out_avals[i].shape)[c]
             for i, name in enumerate(out_names)}
            for c in range(N_CORES)
        ]

    def run(in_maps):
        return run_concat({
            name: np.concatenate([np.asarray(m[name]) for m in in_maps], axis=0)
            for name in in_names
        })

    run.run_concat = run_concat
    return run


def _runner():
    if "runner" not in _CACHE:
        _CACHE["runner"] = _make_runner(_build())
    return _CACHE["runner"]


def _in_maps(pred_hr, target_hr, lr_ref):
    mT = np.ascontiguousarray(_resize_matrix(H, TH).T)  # [512, 128]
    negI = np.ascontiguousarray(-np.eye(TH, dtype=np.float32))
    maps = []
    for i in range(N_CORES):
        sl = slice(i * BS, (i + 1) * BS)
        maps.append({
            "pred": np.ascontiguousarray(pred_hr[sl], dtype=np.float32),
            "targ": np.ascontiguousarray(target_hr[sl], dtype=np.float32),
            "lr": np.ascontiguousarray(lr_ref[sl], dtype=np.float32),
            "mT": mT,
            "negI": negI,
        })
    return maps


def kernel(pred_hr, target_hr, lr_ref, scale):
    assert pred_hr.shape == (B, C, H, W) and target_hr.shape == (B, C, H, W)
    assert lr_ref.shape == (B, C, TH, TW)
    # the batch axis is the shard axis, so the full arrays already ARE the
    # per-core shards stacked along axis 0
    mT = np.ascontiguousarray(_resize_matrix(H, TH).T)  # [512, 128]
    try:
        results = _runner().run_concat({
            "pred": np.asarray(pred_hr, dtype=np.float32),
            "targ": np.asarray(target_hr, dtype=np.float32),
            "lr": np.asarray(lr_ref, dtype=np.float32),
            "mT": np.concatenate([mT] * N_CORES, axis=0),
            "negI": np.concatenate(
                [-np.eye(TH, dtype=np.float32)] * N_CORES, axis=0),
        })
    except Exception:
        # fallback: the stock (uncached) dispatch path
        _CACHE.pop("runner", None)
        res = run_bass_kernel_spmd(_build(), _in_maps(pred_hr, target_hr, lr_ref),
                                   list(range(N_CORES)))
        results = res.results

    nplus, nminus = _plan_cols(TAIL_PLANS[DEFAULT_OPTS["tail_plan"]])
    pix_sum = 0.0
    lr_sum = 0.0
    for i in range(N_CORES):
        a = results[i]["acc_out"].astype(np.float64)
        pix_sum += a[:, :nplus].sum() - a[:, nplus:nplus + nminus].sum()
        lr_sum += a[:, nplus + nminus:].sum()

    pix = np.float32(pix_sum / (B * C * H * W))
    lr_term = np.float32(lr_sum / (B * C * TH * TW))
    pair_term = np.float32(0.0)
    consist = np.float32(LAM_LR * lr_term + pair_term)
    total = np.float32(pix + LAM_CONSIST * consist)
    return (total, pix, consist, lr_term, pair_term)
